# revision 1
# baseline (speedup 1.0000x reference)
"""Trainium2 Bass kernel for nn_CurvatureLoss (retrieval_knn).

Reference math (per batch b of 2, N=8192 points, 3 coords):
  warped = pc_source + pred_flow
  curv2  = curvature(pc_target)              # self-KNN k=10, radius-masked
  moved  = curvature(pc_source, warped)      # self-KNN on source, gather warped
  inter  = interp(warped, pc_target, curv2)  # KNN k=5, inverse-distance weights
  loss   = mean_b sum_i ||inter_i - moved_i||^2

Sharding: 8 cores = 2 batches x 4 query-quarters (2048 query rows each).
curv2 is all-gathered across the 4 cores of each batch.

Device algorithm per 128-query tile (refs = full 8192):
  P[q,j] = 2 Q.R_j - r2_j  (augmented K=4 fp32 matmul, 4x tile_position packed;
      top-k of true negdist D' = P - q2_q is shift-invariant, so q2 is applied
      only to thresholds/weights on [128,8] tiles)
  k=10 self-KNN: max8 -> chunked match_replace+max8 -> v10; thr=max(v10,q2-2.5);
      W = (P >= thr) bf16; PE-transpose W; S-matmul sum_j W^T[j,q]*[T_hi|T_lo|1]
      -> neighbor-sum + count; curv = (S - count*T_q)/9.   (no index gathers)
  k=5 interp: max8 + max_index -> exact top-5 values+indices; weights on [128,8]
      tiles; gather of curv2 via two-level one-hot (j = hi*128+lo) small PE
      matmuls against C~[l, c, h].
"""

import os
import numpy as np

N = 8192          # points per batch
B = 2             # batches
NCORES = 8
QPC = 2048        # query rows per core
RADIUS = 2.5
NEG_BIG = -1e30


def _build(nt=16, debug=0, stages=7):
    """Build the SPMD bass program. nt = number of 128-query tiles per core
    (16 = full 2048 rows). debug adds extra DRAM outputs."""
    import concourse.bacc as bacc
    import concourse.mybir as mybir
    import concourse.tile as tile
    from concourse.masks import make_identity

    f32 = mybir.dt.float32
    bf16 = mybir.dt.bfloat16
    i32 = mybir.dt.int32
    u32 = mybir.dt.uint32
    Alu = mybir.AluOpType
    Act = mybir.ActivationFunctionType
    X = mybir.AxisListType.X
    CAX = mybir.AxisListType.C

    CH = N // 128          # 64 ref chunks of 128
    GK = N // 512          # 16 moving chunks of 512
    NQT = QPC // 128       # 16 query tiles max

    nc = bacc.Bacc(None, num_devices=NCORES)

    tgt_all = nc.dram_tensor("tgt_all", [N, 3], f32, kind="ExternalInput")
    src_all = nc.dram_tensor("src_all", [N, 3], f32, kind="ExternalInput")
    flow_all = nc.dram_tensor("flow_all", [N, 3], f32, kind="ExternalInput")
    tgt_q = nc.dram_tensor("tgt_q", [QPC, 3], f32, kind="ExternalInput")
    src_q = nc.dram_tensor("src_q", [QPC, 3], f32, kind="ExternalInput")
    flow_q = nc.dram_tensor("flow_q", [QPC, 3], f32, kind="ExternalInput")
    loss_out = nc.dram_tensor("loss", [1, 1], f32, kind="ExternalOutput")
    dbg = {}
    if debug:
        for nm in ("curv2_part", "moved_part", "inter_part"):
            dbg[nm] = nc.dram_tensor(nm, [QPC, 3], f32, kind="ExternalOutput")
        dbg["v16_dbg"] = nc.dram_tensor("v16_dbg", [QPC, 16], f32,
                                        kind="ExternalOutput")

    with tile.TileContext(nc) as tc:
        with (
            tc.tile_pool(name="persist", bufs=1) as pers,
            tc.tile_pool(name="dram", bufs=1, space="DRAM") as dram,
            tc.tile_pool(name="pP", bufs=2) as pP,
            tc.tile_pool(name="pP2", bufs=1) as pP2,
            tc.tile_pool(name="pW", bufs=2) as pW,
            tc.tile_pool(name="pWT", bufs=2) as pWT,
            tc.tile_pool(name="small", bufs=4) as small,
            tc.tile_pool(name="psD", bufs=1, space="PSUM") as psD,
            tc.tile_pool(name="psS", bufs=1, space="PSUM") as psS,
            tc.tile_pool(name="psT", bufs=2, space="PSUM") as psT,
            tc.tile_pool(name="psK3", bufs=1, space="PSUM") as psK3,
        ):
            # ---------------- constants ----------------
            ident_bf = pers.tile([128, 128], bf16)
            make_identity(nc, ident_bf)
            ident_f32 = pers.tile([128, 128], f32)
            make_identity(nc, ident_f32)
            iota128 = pers.tile([128, 128], f32)
            nc.gpsimd.iota(iota128[:], pattern=[[1, 128]], base=0,
                           channel_multiplier=0,
                           allow_small_or_imprecise_dtypes=True)
            iota192 = pers.tile([128, 3, 64], f32)
            nc.gpsimd.iota(iota192[:], pattern=[[0, 3], [1, 64]], base=0,
                           channel_multiplier=0,
                           allow_small_or_imprecise_dtypes=True)
            ones3 = pers.tile([3, 1], f32)
            nc.vector.memset(ones3[:], 1.0)
            ones128 = pers.tile([128, 1], f32)
            nc.vector.memset(ones128[:], 1.0)
            neghalf = pers.tile([1, QPC], f32)
            nc.vector.memset(neghalf[:], -0.5)

            # ---------------- wide loads [p, ci, c] (row n = ci*128+p) --------
            def load_wide(name, src, ncol):
                t = pers.tile([128, ncol, 3], f32, tag=name)
                nc.sync.dma_start(t[:], src.rearrange("(ci p) c -> p ci c", p=128))
                return t

            tgtW = load_wide("tgtW", tgt_all, CH)
            srcW = load_wide("srcW", src_all, CH)
            flowW = load_wide("flowW", flow_all, CH)
            warpW = pers.tile([128, CH, 3], f32)
            nc.vector.tensor_add(warpW[:], srcW[:], flowW[:])

            tgt_qW = load_wide("tgt_qW", tgt_q, NQT)
            src_qW = load_wide("src_qW", src_q, NQT)
            flow_qW = load_wide("flow_qW", flow_q, NQT)
            warp_qW = pers.tile([128, NQT, 3], f32)
            nc.vector.tensor_add(warp_qW[:], src_qW[:], flow_qW[:])

            # per-query r2 (q2 lookups): [128, NQT]
            def r2_of(w, ncol, name):
                sq = small.tile([128, ncol, 3], f32, tag="r2sq")
                nc.vector.tensor_mul(sq[:], w[:], w[:])
                r2 = pers.tile([128, ncol], f32, tag=name)
                nc.vector.tensor_reduce(r2[:], sq[:], X, Alu.add)
                return r2

            r2_tgt_q = r2_of(tgt_qW, NQT, "r2_tgt_q")
            r2_src_q = r2_of(src_qW, NQT, "r2_src_q")
            r2_warp_q = r2_of(warp_qW, NQT, "r2_warp_q")

            def minus_r(r2w, name):
                t = pers.tile([128, NQT], f32, tag=name)
                nc.vector.tensor_scalar_add(t[:], r2w[:], -RADIUS)
                return t

            q2m_tgt = minus_r(r2_tgt_q, "q2m_tgt")
            q2m_src = minus_r(r2_src_q, "q2m_src")

            # hi/lo bf16 gather sources: [p, ci, 8] = [hi xyz, 1, lo xyz, 0]
            def make_hl(w, name):
                hl = pers.tile([128, CH, 8], bf16, tag=name)
                nc.vector.tensor_copy(hl[:, :, 0:3], w[:])
                nc.vector.memset(hl[:, :, 3:4], 1.0)
                lo = small.tile([128, CH, 3], f32, tag="hl_lo")
                nc.vector.tensor_sub(lo[:], w[:], hl[:, :, 0:3])
                nc.vector.tensor_copy(hl[:, :, 4:7], lo[:])
                nc.vector.memset(hl[:, :, 7:8], 0.0)
                return hl

            tgtHL = make_hl(tgtW, "tgtHL")
            warpHL = make_hl(warpW, "warpHL")

            # R~^T replicated at partition offsets 32i: rows 32i+0..2 = ref^T,
            # row 32i+3 = r2_ref
            def fill_RT(rt, all_dram):
                for i in range(4):
                    nc.sync.dma_start(rt[32 * i:32 * i + 3, :],
                                      all_dram.rearrange("n c -> c n"))
                sqT = pP.tile([3, N], f32, tag="P")
                nc.vector.tensor_mul(sqT[:], rt[0:3, :], rt[0:3, :])
                for g in range(GK):
                    pr = psS.tile([1, 512], f32, tag="psumS")
                    nc.tensor.matmul(pr[:], ones3[:],
                                     sqT[:, g * 512:(g + 1) * 512],
                                     start=True, stop=True)
                    r2row = small.tile([1, 512], f32, tag="r2row")
                    nc.scalar.activation(r2row[:], pr[:], Act.Copy)
                    nc.sync.dma_start(rt[3:4, g * 512:(g + 1) * 512], r2row[:])
                for i in range(1, 4):
                    nc.sync.dma_start(rt[32 * i + 3:32 * i + 4, :], rt[3:4, :])

            RT_tgt = pers.tile([128, N], f32)
            fill_RT(RT_tgt, tgt_all)
            RT_src = pers.tile([128, N], f32)
            fill_RT(RT_src, src_all)

            # Q~^T shared slot, reloaded per KNN: rows 32i+0..2 = q^T,
            # row 32i+3 = -0.5  (P_psum = Q.R - 0.5 r2; ACT evac scale=2)
            QT = pers.tile([128, QPC], f32)

            def fill_QT(q_dram, add_dram=None):
                nc.sync.dma_start(QT[0:3, :], q_dram.rearrange("n c -> c n"))
                if add_dram is not None:
                    tmpT = pP2.tile([3, QPC], f32, tag="P2")
                    nc.sync.dma_start(tmpT[:], add_dram.rearrange("n c -> c n"))
                    nc.vector.tensor_add(QT[0:3, :], QT[0:3, :], tmpT[:])
                for i in range(1, 4):
                    nc.sync.dma_start(QT[32 * i:32 * i + 3, :], QT[0:3, :])
                for i in range(4):
                    nc.sync.dma_start(QT[32 * i + 3:32 * i + 4, :], neghalf[:])

            # all-gather buffers for curv2
            ag_in = dram.tile([QPC, 3], f32)
            ag_out = dram.tile([N, 3], f32)

            moved_acc = pers.tile([128, NQT, 3], f32)
            loss_acc = pers.tile([128, NQT], f32)

            # ---------------- D' matmul ----------------
            def emit_P(RT_rep, t):
                P = pP.tile([128, GK, 512], f32, tag="P")
                for g in range(GK // 4):
                    pd = psD.tile([128, 4, 512], f32, tag="psumD")
                    for i in range(4):
                        c = 4 * g + i
                        nc.tensor.matmul(
                            pd[:, i, :],
                            QT[32 * i:32 * i + 4, t * 128:(t + 1) * 128],
                            RT_rep[32 * i:32 * i + 4, c * 512:(c + 1) * 512],
                            start=True, stop=True,
                            tile_position=(32 * i, 0),
                        )
                    nc.scalar.activation(P[:, 4 * g:4 * (g + 1), :], pd[:],
                                         Act.Copy, scale=2.0)
                return P

            # ---------------- self-KNN k=10 curvature ----------------
            def emit_curv(RT_rep, q2mW, hlW, centerW, t, out_cb):
                P = emit_P(RT_rep, t)
                Pf = P[:].rearrange("p a b -> p (a b)")
                cand = small.tile([128, 8, 8], f32, tag="cand")
                for h in range(8):
                    nc.vector.max(cand[:, h, :], Pf[:, h * 1024:(h + 1) * 1024])
                candf = cand[:].rearrange("p a b -> p (a b)")
                v8 = small.tile([128, 8], f32, tag="v8")
                nc.vector.max(v8[:], candf)
                cand2 = small.tile([128, 64], f32, tag="cand2")
                nc.vector.match_replace(cand2[:], v8[:], candf, NEG_BIG)
                v16 = small.tile([128, 8], f32, tag="v16")
                nc.vector.max(v16[:], cand2[:])
                thr = small.tile([128, 1], f32, tag="thr")
                nc.vector.tensor_tensor(thr[:], v16[:, 1:2], q2mW[:, t:t + 1],
                                        Alu.max)
                ps = psS.tile([128, 8], f32, tag="psumS")
                for h in range(2):
                    Wt = pW.tile([128, 4096], bf16, tag="W")
                    nc.gpsimd.tensor_scalar(Wt[:], Pf[:, h * 4096:(h + 1) * 4096],
                                            thr[:], None, Alu.is_ge)
                    for g in range(4):
                        pt = psT.tile([128, 8, 128], bf16, tag="psumT")
                        for j in range(8):
                            cl = (8 * g + j)          # chunk within half
                            c = 32 * h + cl           # global ref chunk
                            nc.tensor.transpose(pt[:, j, :],
                                                Wt[:, cl * 128:(cl + 1) * 128],
                                                ident_bf[:])
                        WT = pWT.tile([128, 8, 128], bf16, tag="WT")
                        nc.scalar.activation(WT[:], pt[:], Act.Copy)
                        for j in range(8):
                            c = 32 * h + 8 * g + j
                            nc.tensor.matmul(ps[:], WT[:, j, :], hlW[:, c, :],
                                             start=(c == 0), stop=(c == CH - 1))
                S8 = small.tile([128, 8], f32, tag="S8")
                nc.scalar.activation(S8[:], ps[:], Act.Copy)
                S4 = small.tile([128, 4], f32, tag="S4")
                nc.vector.tensor_add(S4[:], S8[:, 0:4], S8[:, 4:8])
                curv = small.tile([128, 3], f32, tag="curv")
                nc.vector.scalar_tensor_tensor(
                    curv[:], centerW[:, t, :], S4[:, 3:4], S4[:, 0:3],
                    Alu.mult, Alu.subtract)
                nc.vector.tensor_scalar_mul(curv[:], curv[:], -1.0 / 9.0)
                out_cb(curv, v8, v16)

            # ---------------- KNN1: curvature of target ----------------
            fill_QT(tgt_q)
            for t in range(nt if stages & 1 else 0):
                def cb1(curv, v8, v16, t=t):
                    nc.sync.dma_start(ag_in[t * 128:(t + 1) * 128, :], curv[:])
                    if debug:
                        nc.sync.dma_start(
                            dbg["curv2_part"][t * 128:(t + 1) * 128, :], curv[:])
                        nc.sync.dma_start(
                            dbg["v16_dbg"][t * 128:(t + 1) * 128, 0:8], v8[:])
                        nc.sync.dma_start(
                            dbg["v16_dbg"][t * 128:(t + 1) * 128, 8:16], v16[:])
                emit_curv(RT_tgt, q2m_tgt, tgtHL, tgt_qW, t, cb1)
            if nt < NQT:
                zc = small.tile([128, 3], f32, tag="curv")
                nc.vector.memset(zc[:], 0.0)
                for t in range(nt, NQT):
                    nc.sync.dma_start(ag_in[t * 128:(t + 1) * 128, :], zc[:])

            nc.gpsimd.collective_compute(
                "AllGather", Alu.bypass,
                replica_groups=[[0, 1, 2, 3], [4, 5, 6, 7]],
                ins=[ag_in.opt()], outs=[ag_out.opt()])

            # ---------------- KNN2: curvature of source (gather warped) -------
            fill_QT(src_q)
            for t in range(nt if stages & 2 else 0):
                def cb2(curv, v8, v16, t=t):
                    nc.vector.tensor_copy(moved_acc[:, t, :], curv[:])
                    if debug:
                        nc.sync.dma_start(
                            dbg["moved_part"][t * 128:(t + 1) * 128, :], curv[:])
                emit_curv(RT_src, q2m_src, warpHL, warp_qW, t, cb2)

            K3LVL = int(os.environ.get("K3LVL", "4"))
            # gather table C~[l, c, h] = curv2[h*128+l, c]
            Ctab = pers.tile([128, 3, CH], f32)
            if K3LVL >= 4:
                nc.sync.dma_start(Ctab[:],
                                  ag_out[:].rearrange("(h l) c -> l c h", l=128))
            else:
                nc.vector.memset(Ctab[:], 0.25)

            # ---------------- KNN3: interp k=5 ----------------
            if int(os.environ.get("K3QT", "1")):
                fill_QT(src_q, add_dram=flow_q)   # warped queries
            else:
                fill_QT(src_q)
            for t in range(nt if stages & 4 else 0):
                P = emit_P(RT_tgt, t)
                Pf = P[:].rearrange("p a b -> p (a b)")
                cand = small.tile([128, 8, 8], f32, tag="cand")
                for h in range(8):
                    nc.vector.max(cand[:, h, :], Pf[:, h * 1024:(h + 1) * 1024])
                v8 = small.tile([128, 8], f32, tag="v8")
                nc.vector.max(v8[:], cand[:].rearrange("p a b -> p (a b)"))
                if K3LVL >= 2:
                    idx = small.tile([128, 8], u32, tag="idx")
                    nc.vector.max_index(idx[:], v8[:], Pf)
                    lo_i = small.tile([128, 8], u32, tag="lo_i")
                    hi_i = small.tile([128, 8], u32, tag="hi_i")
                    nc.vector.tensor_scalar(lo_i[:], idx[:], 127, None,
                                            Alu.bitwise_and)
                    nc.vector.tensor_scalar(hi_i[:], idx[:], 7, None,
                                            Alu.logical_shift_right)
                    lo = small.tile([128, 8], f32, tag="lo")
                    hi = small.tile([128, 8], f32, tag="hi")
                    nc.vector.tensor_copy(lo[:], lo_i[:])
                    nc.vector.tensor_copy(hi[:], hi_i[:])
                if K3LVL >= 3:
                    d8 = small.tile([128, 8], f32, tag="d8")
                    nc.vector.tensor_scalar(d8[:], v8[:], r2_warp_q[:, t:t + 1],
                                            -1.0, Alu.subtract, Alu.mult)
                    w5 = small.tile([128, 5], f32, tag="w5")
                    nc.vector.tensor_scalar_add(w5[:], d8[:, 0:5], 1e-8)
                    nc.vector.reciprocal(w5[:], w5[:])
                    inrad = small.tile([128, 5], f32, tag="inrad")
                    nc.vector.tensor_scalar(inrad[:], d8[:, 0:5], RADIUS, None,
                                            Alu.is_le)
                    w_in = small.tile([128, 5], f32, tag="w_in")
                    nc.vector.tensor_mul(w_in[:], w5[:], inrad[:])
                    wsum = small.tile([128, 2], f32, tag="wsum")
                    nc.vector.tensor_reduce(wsum[:, 0:1], w5[:], X, Alu.add)
                    nc.vector.tensor_reduce(wsum[:, 1:2], w_in[:], X, Alu.add)
                    winv = small.tile([128, 1], f32, tag="winv")
                    nc.vector.reciprocal(winv[:], wsum[:, 0:1])
                    alpha0 = small.tile([128, 1], f32, tag="alpha0")
                    nc.vector.tensor_sub(alpha0[:], wsum[:, 0:1], wsum[:, 1:2])
                    nc.vector.tensor_add(alpha0[:], alpha0[:], w_in[:, 0:1])
                acc = small.tile([128, 3], f32, tag="acc3")
                if K3LVL >= 4:
                    for k in range(5):
                        Bk = small.tile([128, 128], f32, tag="Bk")
                        nc.vector.tensor_scalar(Bk[:], iota128[:], lo[:, k:k + 1],
                                                None, Alu.is_equal)
                        A3 = small.tile([128, 3, 64], f32, tag="A3")
                        nc.vector.tensor_scalar(A3[:], iota192[:], hi[:, k:k + 1],
                                                None, Alu.is_equal)
                        pb = psK3.tile([128, 128], f32, tag="bm")
                        nc.tensor.transpose(pb[:], Bk[:], ident_f32[:])
                        BT = small.tile([128, 128], f32, tag="BT")
                        nc.scalar.activation(BT[:], pb[:], Act.Copy)
                        pm = psK3.tile([128, 3, 64], f32, tag="bm")
                        nc.tensor.matmul(pm[:], BT[:], Ctab[:], start=True,
                                         stop=True)
                        g3 = small.tile([128, 3, 64], f32, tag="g3")
                        nc.vector.tensor_mul(g3[:], pm[:], A3[:])
                        gk = small.tile([128, 3], f32, tag="gk")
                        nc.vector.tensor_reduce(gk[:], g3[:], X, Alu.add)
                        wk = alpha0[:] if k == 0 else w_in[:, k:k + 1]
                        if k == 0:
                            nc.vector.tensor_scalar(acc[:], gk[:], wk, None,
                                                    Alu.mult)
                        else:
                            nc.vector.scalar_tensor_tensor(acc[:], gk[:], wk,
                                                           acc[:], Alu.mult,
                                                           Alu.add)
                elif K3LVL >= 3:
                    nc.vector.tensor_scalar(acc[:], v8[:, 0:3], winv[:], None,
                                            Alu.mult)
                elif K3LVL >= 2:
                    nc.vector.tensor_copy(acc[:], lo[:, 0:3])
                else:
                    nc.vector.tensor_copy(acc[:], v8[:, 0:3])
                winv2 = winv[:] if K3LVL >= 3 else None
                inter = small.tile([128, 3], f32, tag="inter")
                if winv2 is not None:
                    nc.vector.tensor_scalar(inter[:], acc[:], winv2, None,
                                            Alu.mult)
                else:
                    nc.vector.tensor_copy(inter[:], acc[:])
                if debug:
                    nc.sync.dma_start(dbg["inter_part"][t * 128:(t + 1) * 128, :],
                                      inter[:])
                diff = small.tile([128, 3], f32, tag="diff")
                nc.vector.tensor_sub(diff[:], inter[:], moved_acc[:, t, :])
                sqd = small.tile([128, 3], f32, tag="sqd")
                if int(os.environ.get("K3TTR", "0")):
                    nc.vector.tensor_tensor_reduce(
                        sqd[:], diff[:], diff[:], 1.0, 0.0,
                        Alu.mult, Alu.add, loss_acc[:, t:t + 1])
                else:
                    nc.vector.tensor_mul(sqd[:], diff[:], diff[:])
                    nc.vector.tensor_reduce(loss_acc[:, t:t + 1], sqd[:], X,
                                            Alu.add)

            if nt < NQT:
                nc.vector.memset(loss_acc[:, nt:NQT], 0.0)
            pl = psS.tile([1, NQT], f32, tag="psumS")
            nc.tensor.matmul(pl[:], ones128[:], loss_acc[:], start=True, stop=True)
            lsum = pers.tile([1, NQT], f32)
            nc.scalar.activation(lsum[:], pl[:], Act.Copy)
            ltot = pers.tile([1, 1], f32)
            nc.vector.tensor_reduce(ltot[:], lsum[:], X, Alu.add)
            nc.sync.dma_start(loss_out[:], ltot[:])

    nc.compile()
    return nc


def _shard_inputs(pc_source, pc_target, pred_flow):
    maps = []
    for c in range(NCORES):
        b, qi = divmod(c, 4)
        sl = slice(qi * QPC, (qi + 1) * QPC)
        maps.append({
            "tgt_all": np.ascontiguousarray(pc_target[b]),
            "src_all": np.ascontiguousarray(pc_source[b]),
            "flow_all": np.ascontiguousarray(pred_flow[b]),
            "tgt_q": np.ascontiguousarray(pc_target[b][sl]),
            "src_q": np.ascontiguousarray(pc_source[b][sl]),
            "flow_q": np.ascontiguousarray(pred_flow[b][sl]),
        })
    return maps


_CACHED = {}


def _get_program(nt=16, debug=0):
    key = (nt, debug)
    if key not in _CACHED:
        _CACHED[key] = _build(nt, debug)
    return _CACHED[key]


def kernel(pc_source, pc_target, pred_flow):
    from concourse.bass_utils import run_bass_kernel_spmd

    pc_source = np.asarray(pc_source, dtype=np.float32)
    pc_target = np.asarray(pc_target, dtype=np.float32)
    pred_flow = np.asarray(pred_flow, dtype=np.float32)
    nc = _get_program()
    in_maps = _shard_inputs(pc_source, pc_target, pred_flow)
    res = run_bass_kernel_spmd(nc, in_maps, core_ids=list(range(NCORES)))
    total = np.float32(0.0)
    for c in range(NCORES):
        total += res.results[c]["loss"][0, 0]
    return np.asarray(np.float32(total / B))

